# revision 2
# baseline (speedup 1.0000x reference)
"""Trainium2 Bass kernel for nn_AttentionLayer (B=64, L1=L2=512, H=A=1024).

Math (per batch b):
    P_lt = tanh(reps_lt[b] @ W) * diag_W        [L1, A]
    P_rt = tanh(reps_rt[b] @ W)                 [L2, A]
    S    = P_lt @ P_rt.T                        [L1, L2]
    out  = softmax(S, axis=-1)                  (masks are all-ones -> no-ops)

Distribution: data-parallel over batch across 8 NeuronCores (8 batches/core).

The kernel is PE-streaming-bound (~1280 matmuls x 512 moving columns =
655K cycles @ 2.4 GHz warm = 273 us); everything else overlaps.  This
version squeezes the head/tail latency:
  - W and X are DMA'd in per-k-chunk contiguous tiles so the first
    matmul only waits for ~384 KB, not 3 MB;
  - ~3.4 us of dummy matmuls run during the DMA head so the PE HAM
    clock-gate is already at 2.4 GHz when real work starts;
  - softmax skips the max-subtraction (|S| <= sum|diag_W| ~ 26, so
    exp cannot overflow fp32) shortening the per-chunk chain;
  - output DMA is chunked per 128-row block instead of per batch.

Layout: the PE contracts over the partition dim, so both matmuls want
[contraction, free] operands.  reps are transposed to [H, L] on the host
during sharding; projections then produce P.T in [A, L] layout directly,
and the scores matmul consumes those tiles with no on-device transposes.
"""

from contextlib import ExitStack

import numpy as np

import concourse.bass as bass
import concourse.bacc as bacc
import concourse.mybir as mybir
import concourse.tile as tile
from concourse.bass_utils import run_bass_kernel_spmd

B, L, H, A = 64, 512, 1024, 1024
NCORES = 8
BPC = B // NCORES  # batches per core
PD = 128  # partition dim
KC = H // PD  # contraction chunks for proj
MC = A // PD  # att-dim chunks
LC = L // PD  # L1 chunks for scores

F32 = mybir.dt.float32
XDT = mybir.dt.bfloat16
WARMUP_MMS = 40  # ~40 x 107ns cold ~= 4.3us > 3.4us HAM window


def _build_body(ctx: ExitStack, tc: "tile.TileContext", out, xt_lt, xt_rt, w, dw,
                repeat: int = 1):
    nc = tc.nc
    act = mybir.ActivationFunctionType

    wp = ctx.enter_context(tc.tile_pool(name="wpool", bufs=1))
    xp = ctx.enter_context(tc.tile_pool(name="xpool", bufs=2))
    pp = ctx.enter_context(tc.tile_pool(name="ppool", bufs=2))
    ep = ctx.enter_context(tc.tile_pool(name="epool", bufs=3))
    sp = ctx.enter_context(tc.tile_pool(name="spool", bufs=6))
    op = ctx.enter_context(tc.tile_pool(name="opool", bufs=4))
    ps_warm = ctx.enter_context(tc.tile_pool(name="psW", bufs=1, space="PSUM"))
    ps_proj = ctx.enter_context(tc.tile_pool(name="psA", bufs=4, space="PSUM"))
    ps_scr = ctx.enter_context(tc.tile_pool(name="psB", bufs=3, space="PSUM"))

    # PE warm-up: dummy matmuls on a zeroed tile, issued before any DMA
    # dependency so they run during the input-DMA head and lift the HAM
    # clock gate to 2.4 GHz.  Results land in a reserved PSUM bank that
    # nothing reads.
    warm = wp.tile([PD, PD], XDT)
    nc.vector.memset(warm, 0.0)
    wps = ps_warm.tile([PD, PD], F32)
    for i in range(WARMUP_MMS):
        nc.tensor.matmul(wps, lhsT=warm, rhs=warm, start=True, stop=True)

    # Per-k weight tiles: w_k[p, a] = W[k*128 + p, a]  (contiguous DMA).
    w_sb = []
    for k in range(KC):
        wk = wp.tile([PD, A], XDT, tag=f"w{k}")
        nc.sync.dma_start(out=wk, in_=w[k * PD:(k + 1) * PD, :])
        w_sb.append(wk)
    # Per-partition diagonal scale: dw_sb[p, m] = dw[m*128 + p]
    dw_sb = wp.tile([PD, MC], F32)
    nc.sync.dma_start(out=dw_sb, in_=dw.rearrange("(m p) -> p m", p=PD))

    for b in [bb for _ in range(repeat) for bb in range(BPC)]:
        # Per-k x tiles (contiguous [128, 512] DMAs).
        x_lt = []
        x_rt = []
        for k in range(KC):
            xl = xp.tile([PD, L], XDT, tag=f"xlt{k}")
            nc.sync.dma_start(out=xl, in_=xt_lt[b, k * PD:(k + 1) * PD, :])
            x_lt.append(xl)
            xr = xp.tile([PD, L], XDT, tag=f"xrt{k}")
            nc.sync.dma_start(out=xr, in_=xt_rt[b, k * PD:(k + 1) * PD, :])
            x_rt.append(xr)

        # Projections -> P.T tiles in [A, L] layout (m-chunk per tile).
        lt_p = []
        rt_p = []
        for m in range(MC):
            ps = ps_proj.tile([PD, L], F32, tag="psp")
            for k in range(KC):
                nc.tensor.matmul(
                    ps,
                    lhsT=w_sb[k][:, m * PD:(m + 1) * PD],
                    rhs=x_lt[k],
                    start=(k == 0),
                    stop=(k == KC - 1),
                )
            # tanh in-place on PSUM, then scale by diag_W into SBUF.
            nc.scalar.activation(ps, ps, act.Tanh)
            ltm = pp.tile([PD, L], XDT, tag=f"lt{m}")
            nc.vector.tensor_scalar_mul(ltm, ps, dw_sb[:, m:m + 1])
            lt_p.append(ltm)
        for m in range(MC):
            ps = ps_proj.tile([PD, L], F32, tag="psp")
            for k in range(KC):
                nc.tensor.matmul(
                    ps,
                    lhsT=w_sb[k][:, m * PD:(m + 1) * PD],
                    rhs=x_rt[k],
                    start=(k == 0),
                    stop=(k == KC - 1),
                )
            rtm = pp.tile([PD, L], XDT, tag=f"rt{m}")
            nc.scalar.activation(rtm, ps, act.Tanh)
            rt_p.append(rtm)

        # Scores + row softmax (no max-subtraction: |S| <= sum|dw| ~ 26).
        for lm in range(LC):
            ps2 = ps_scr.tile([PD, L], F32, tag="pss")
            for m in range(MC):
                nc.tensor.matmul(
                    ps2,
                    lhsT=lt_p[m][:, lm * PD:(lm + 1) * PD],
                    rhs=rt_p[m],
                    start=(m == 0),
                    stop=(m == MC - 1),
                )
            ex = ep.tile([PD, L], F32, tag="ex")
            sm = sp.tile([PD, 1], F32, tag="sm")
            nc.scalar.activation(ex, ps2, act.Exp, accum_out=sm)
            rc = sp.tile([PD, 1], F32, tag="rc")
            nc.vector.reciprocal(rc, sm)
            o_sb = op.tile([PD, L], F32, tag="o")
            nc.vector.tensor_scalar_mul(o_sb, ex, rc)
            nc.sync.dma_start(out=out[b, lm * PD:(lm + 1) * PD, :], in_=o_sb)


def build_nc(repeat: int = 1) -> "bacc.Bacc":
    nc = bacc.Bacc("TRN2", target_bir_lowering=False, debug=False, num_devices=NCORES)
    xt_lt = nc.dram_tensor("xt_lt", [BPC, H, L], XDT, kind="ExternalInput").ap()
    xt_rt = nc.dram_tensor("xt_rt", [BPC, H, L], XDT, kind="ExternalInput").ap()
    w = nc.dram_tensor("w", [H, A], XDT, kind="ExternalInput").ap()
    dw = nc.dram_tensor("dw", [A], F32, kind="ExternalInput").ap()
    out = nc.dram_tensor("out", [BPC, L, L], F32, kind="ExternalOutput").ap()
    with tile.TileContext(nc) as tc, ExitStack() as ctx:
        _build_body(ctx, tc, out, xt_lt, xt_rt, w, dw, repeat=repeat)
    nc.compile()
    return nc


_NC_CACHE = None


def _get_nc():
    global _NC_CACHE
    if _NC_CACHE is None:
        _NC_CACHE = build_nc()
    return _NC_CACHE


def _x_np(a):
    import ml_dtypes

    return np.ascontiguousarray(a).astype(ml_dtypes.bfloat16)


def make_in_maps(reps_lt, reps_rt, attn_w1, diagonal_W):
    """Shard + lay out host-side: per-core [BPC, H, L] transposed inputs."""
    w = _x_np(np.asarray(attn_w1, dtype=np.float32))
    dw = np.ascontiguousarray(np.asarray(diagonal_W, dtype=np.float32).reshape(A))
    in_maps = []
    for c in range(NCORES):
        sl = slice(c * BPC, (c + 1) * BPC)
        xt_lt = _x_np(np.asarray(reps_lt[sl], dtype=np.float32).transpose(0, 2, 1))
        xt_rt = _x_np(np.asarray(reps_rt[sl], dtype=np.float32).transpose(0, 2, 1))
        in_maps.append({"xt_lt": xt_lt, "xt_rt": xt_rt, "w": w, "dw": dw})
    return in_maps


def kernel(reps_lt, reps_rt, mask_lt, mask_rt, attn_w1, diagonal_W):
    reps_lt = np.asarray(reps_lt, dtype=np.float32)
    reps_rt = np.asarray(reps_rt, dtype=np.float32)
    mask_lt = np.asarray(mask_lt, dtype=np.float32)
    mask_rt = np.asarray(mask_rt, dtype=np.float32)
    attn_w1 = np.asarray(attn_w1, dtype=np.float32)
    diagonal_W = np.asarray(diagonal_W, dtype=np.float32)

    if not (np.all(mask_lt == 1.0) and np.all(mask_rt == 1.0)):
        # General-mask fallback (never hit for this problem's all-ones masks):
        # multiplicative masking changes the softmax input, so compute on host.
        attn_lt = np.tanh(reps_lt @ attn_w1) * diagonal_W.reshape(1, 1, A)
        attn_rt = np.tanh(reps_rt @ attn_w1)
        s = np.einsum("bla,bra->blr", attn_lt, attn_rt)
        s = s * mask_lt[:, :, None] * mask_rt[:, None, :]
        e = np.exp(s - s.max(-1, keepdims=True))
        p = e / e.sum(-1, keepdims=True)
        return (p * mask_lt[:, :, None] * mask_rt[:, None, :]).astype(np.float32)

    nc = _get_nc()
    in_maps = make_in_maps(reps_lt, reps_rt, attn_w1, diagonal_W)
    res = run_bass_kernel_spmd(nc, in_maps, core_ids=list(range(NCORES)))
    out = np.concatenate([res.results[c]["out"] for c in range(NCORES)], axis=0)
    return np.ascontiguousarray(out.astype(np.float32))


# revision 3
# speedup vs baseline: 1.1100x; 1.1100x over previous
"""Trainium2 Bass kernel for nn_AttentionLayer (B=64, L1=L2=512, H=A=1024).

Math (per batch b):
    P_lt = tanh(reps_lt[b] @ W) * diag_W        [L1, A]
    P_rt = tanh(reps_rt[b] @ W)                 [L2, A]
    S    = P_lt @ P_rt.T                        [L1, L2]
    out  = softmax(S, axis=-1)                  (masks are all-ones -> no-ops)

Distribution: data-parallel over batch across 8 NeuronCores (8 batches/core).

The kernel is PE-streaming-bound (~1280 matmuls x 512 moving columns =
655K cycles @ 2.4 GHz warm = 273 us); everything else overlaps.  This
version squeezes the head/tail latency:
  - W and X are DMA'd in per-k-chunk contiguous tiles so the first
    matmul only waits for ~384 KB, not 3 MB;
  - ~3.4 us of dummy matmuls run during the DMA head so the PE HAM
    clock-gate is already at 2.4 GHz when real work starts;
  - softmax skips the max-subtraction (|S| <= sum|diag_W| ~ 26, so
    exp cannot overflow fp32) shortening the per-chunk chain;
  - output DMA is chunked per 128-row block instead of per batch.

Layout: the PE contracts over the partition dim, so both matmuls want
[contraction, free] operands.  reps are transposed to [H, L] on the host
during sharding; projections then produce P.T in [A, L] layout directly,
and the scores matmul consumes those tiles with no on-device transposes.
"""

from contextlib import ExitStack

import numpy as np

import concourse.bass as bass
import concourse.bacc as bacc
import concourse.mybir as mybir
import concourse.tile as tile
from concourse.bass_utils import run_bass_kernel_spmd

B, L, H, A = 64, 512, 1024, 1024
NCORES = 8
BPC = B // NCORES  # batches per core
PD = 128  # partition dim
KC = H // PD  # contraction chunks for proj
MC = A // PD  # att-dim chunks
LC = L // PD  # L1 chunks for scores

F32 = mybir.dt.float32
XDT = mybir.dt.bfloat16
WARMUP_MMS = 16  # ~1.7us; chunk-paced real matmuls continue the HAM activity


def _build_body(ctx: ExitStack, tc: "tile.TileContext", out, xt_lt, xt_rt, w, dw,
                repeat: int = 1):
    nc = tc.nc
    act = mybir.ActivationFunctionType

    wp = ctx.enter_context(tc.tile_pool(name="wpool", bufs=1))
    xp = ctx.enter_context(tc.tile_pool(name="xpool", bufs=2))
    pp = ctx.enter_context(tc.tile_pool(name="ppool", bufs=2))
    ep = ctx.enter_context(tc.tile_pool(name="epool", bufs=3))
    sp = ctx.enter_context(tc.tile_pool(name="spool", bufs=6))
    op = ctx.enter_context(tc.tile_pool(name="opool", bufs=4))
    ps_warm = ctx.enter_context(tc.tile_pool(name="psW", bufs=1, space="PSUM"))
    ps_proj = ctx.enter_context(tc.tile_pool(name="psA", bufs=4, space="PSUM"))
    ps_scr = ctx.enter_context(tc.tile_pool(name="psB", bufs=3, space="PSUM"))

    # PE warm-up: dummy matmuls on a zeroed tile, issued before any DMA
    # dependency so they run during the input-DMA head and lift the HAM
    # clock gate to 2.4 GHz.  Results land in a reserved PSUM bank that
    # nothing reads.
    warm = wp.tile([PD, PD], XDT)
    nc.vector.memset(warm, 0.0)
    wps = ps_warm.tile([PD, PD], F32)
    for i in range(WARMUP_MMS):
        nc.tensor.matmul(wps, lhsT=warm, rhs=warm, start=True, stop=True)

    # Head DMA order: (w_k, x_lt_k) pairs so the k-th projection chunk can
    # start as soon as ~((k+1) * 384 KB) has landed; x_rt and dw follow
    # (they are not needed until ~15 us in).
    w_sb = []
    first_xlt = []
    for k in range(KC):
        wk = wp.tile([PD, A], XDT, tag=f"w{k}")
        nc.sync.dma_start(out=wk, in_=w[k * PD:(k + 1) * PD, :])
        w_sb.append(wk)
        xl = xp.tile([PD, L], XDT, tag=f"xlt{k}")
        nc.sync.dma_start(out=xl, in_=xt_lt[0, k * PD:(k + 1) * PD, :])
        first_xlt.append(xl)
    dw_sb = wp.tile([PD, MC], F32)
    nc.sync.dma_start(out=dw_sb, in_=dw.rearrange("(m p) -> p m", p=PD))

    first = True
    for b in [bb for _ in range(repeat) for bb in range(BPC)]:
        # Per-k x tiles (contiguous [128, 512] DMAs); lt before rt so the
        # lt projection's chunk-k dependency clears as early as possible.
        if first:
            x_lt = first_xlt
        else:
            x_lt = []
            for k in range(KC):
                xl = xp.tile([PD, L], XDT, tag=f"xlt{k}")
                nc.sync.dma_start(out=xl, in_=xt_lt[b, k * PD:(k + 1) * PD, :])
                x_lt.append(xl)
        x_rt = []
        for k in range(KC):
            xr = xp.tile([PD, L], XDT, tag=f"xrt{k}")
            nc.sync.dma_start(out=xr, in_=xt_rt[b, k * PD:(k + 1) * PD, :])
            x_rt.append(xr)

        # Projections -> P.T tiles in [A, L] layout (m-chunk per tile).
        lt_p = [None] * MC
        rt_p = []
        if first:
            # k-outer in two half-sweeps of 4 m-chunks (= 4 PSUM banks):
            # each (w_k, x_lt_k) chunk is consumed the moment its DMA lands,
            # so the PE tracks the head DMA instead of waiting for all of W.
            for half in range(2):
                ms = range(half * 4, half * 4 + 4)
                pss = {
                    m: ps_proj.tile([PD, L], F32, tag="psp", name=f"ps_h{half}_{m}")
                    for m in ms
                }
                for k in range(KC):
                    for m in ms:
                        nc.tensor.matmul(
                            pss[m],
                            lhsT=w_sb[k][:, m * PD:(m + 1) * PD],
                            rhs=x_lt[k],
                            start=(k == 0),
                            stop=(k == KC - 1),
                        )
                for m in ms:
                    nc.scalar.activation(pss[m], pss[m], act.Tanh)
                    ltm = pp.tile([PD, L], XDT, tag=f"lt{m}")
                    nc.vector.tensor_scalar_mul(ltm, pss[m], dw_sb[:, m:m + 1])
                    lt_p[m] = ltm
            first = False
        else:
            for m in range(MC):
                ps = ps_proj.tile([PD, L], F32, tag="psp")
                for k in range(KC):
                    nc.tensor.matmul(
                        ps,
                        lhsT=w_sb[k][:, m * PD:(m + 1) * PD],
                        rhs=x_lt[k],
                        start=(k == 0),
                        stop=(k == KC - 1),
                    )
                # tanh in-place on PSUM, then scale by diag_W into SBUF.
                nc.scalar.activation(ps, ps, act.Tanh)
                ltm = pp.tile([PD, L], XDT, tag=f"lt{m}")
                nc.vector.tensor_scalar_mul(ltm, ps, dw_sb[:, m:m + 1])
                lt_p[m] = ltm
        for m in range(MC):
            ps = ps_proj.tile([PD, L], F32, tag="psp")
            for k in range(KC):
                nc.tensor.matmul(
                    ps,
                    lhsT=w_sb[k][:, m * PD:(m + 1) * PD],
                    rhs=x_rt[k],
                    start=(k == 0),
                    stop=(k == KC - 1),
                )
            rtm = pp.tile([PD, L], XDT, tag=f"rt{m}")
            nc.scalar.activation(rtm, ps, act.Tanh)
            rt_p.append(rtm)

        # Scores + row softmax (no max-subtraction: |S| <= sum|dw| ~ 26).
        for lm in range(LC):
            ps2 = ps_scr.tile([PD, L], F32, tag="pss")
            for m in range(MC):
                nc.tensor.matmul(
                    ps2,
                    lhsT=lt_p[m][:, lm * PD:(lm + 1) * PD],
                    rhs=rt_p[m],
                    start=(m == 0),
                    stop=(m == MC - 1),
                )
            ex = ep.tile([PD, L], F32, tag="ex")
            sm = sp.tile([PD, 1], F32, tag="sm")
            nc.scalar.activation(ex, ps2, act.Exp, accum_out=sm)
            rc = sp.tile([PD, 1], F32, tag="rc")
            nc.vector.reciprocal(rc, sm)
            o_sb = op.tile([PD, L], F32, tag="o")
            nc.vector.tensor_scalar_mul(o_sb, ex, rc)
            nc.sync.dma_start(out=out[b, lm * PD:(lm + 1) * PD, :], in_=o_sb)


def build_nc(repeat: int = 1) -> "bacc.Bacc":
    nc = bacc.Bacc("TRN2", target_bir_lowering=False, debug=False, num_devices=NCORES)
    xt_lt = nc.dram_tensor("xt_lt", [BPC, H, L], XDT, kind="ExternalInput").ap()
    xt_rt = nc.dram_tensor("xt_rt", [BPC, H, L], XDT, kind="ExternalInput").ap()
    w = nc.dram_tensor("w", [H, A], XDT, kind="ExternalInput").ap()
    dw = nc.dram_tensor("dw", [A], F32, kind="ExternalInput").ap()
    out = nc.dram_tensor("out", [BPC, L, L], F32, kind="ExternalOutput").ap()
    with tile.TileContext(nc) as tc, ExitStack() as ctx:
        _build_body(ctx, tc, out, xt_lt, xt_rt, w, dw, repeat=repeat)
    nc.compile()
    return nc


_NC_CACHE = None


def _get_nc():
    global _NC_CACHE
    if _NC_CACHE is None:
        _NC_CACHE = build_nc()
    return _NC_CACHE


def _x_np(a):
    import ml_dtypes

    return np.ascontiguousarray(a).astype(ml_dtypes.bfloat16)


def make_in_maps(reps_lt, reps_rt, attn_w1, diagonal_W):
    """Shard + lay out host-side: per-core [BPC, H, L] transposed inputs."""
    w = _x_np(np.asarray(attn_w1, dtype=np.float32))
    dw = np.ascontiguousarray(np.asarray(diagonal_W, dtype=np.float32).reshape(A))
    in_maps = []
    for c in range(NCORES):
        sl = slice(c * BPC, (c + 1) * BPC)
        xt_lt = _x_np(np.asarray(reps_lt[sl], dtype=np.float32).transpose(0, 2, 1))
        xt_rt = _x_np(np.asarray(reps_rt[sl], dtype=np.float32).transpose(0, 2, 1))
        in_maps.append({"xt_lt": xt_lt, "xt_rt": xt_rt, "w": w, "dw": dw})
    return in_maps


def kernel(reps_lt, reps_rt, mask_lt, mask_rt, attn_w1, diagonal_W):
    reps_lt = np.asarray(reps_lt, dtype=np.float32)
    reps_rt = np.asarray(reps_rt, dtype=np.float32)
    mask_lt = np.asarray(mask_lt, dtype=np.float32)
    mask_rt = np.asarray(mask_rt, dtype=np.float32)
    attn_w1 = np.asarray(attn_w1, dtype=np.float32)
    diagonal_W = np.asarray(diagonal_W, dtype=np.float32)

    if not (np.all(mask_lt == 1.0) and np.all(mask_rt == 1.0)):
        # General-mask fallback (never hit for this problem's all-ones masks):
        # multiplicative masking changes the softmax input, so compute on host.
        attn_lt = np.tanh(reps_lt @ attn_w1) * diagonal_W.reshape(1, 1, A)
        attn_rt = np.tanh(reps_rt @ attn_w1)
        s = np.einsum("bla,bra->blr", attn_lt, attn_rt)
        s = s * mask_lt[:, :, None] * mask_rt[:, None, :]
        e = np.exp(s - s.max(-1, keepdims=True))
        p = e / e.sum(-1, keepdims=True)
        return (p * mask_lt[:, :, None] * mask_rt[:, None, :]).astype(np.float32)

    nc = _get_nc()
    in_maps = make_in_maps(reps_lt, reps_rt, attn_w1, diagonal_W)
    res = run_bass_kernel_spmd(nc, in_maps, core_ids=list(range(NCORES)))
    out = np.concatenate([res.results[c]["out"] for c in range(NCORES)], axis=0)
    return np.ascontiguousarray(out.astype(np.float32))


# revision 4
# speedup vs baseline: 1.1260x; 1.0144x over previous
"""Trainium2 Bass kernel for nn_AttentionLayer (B=64, L1=L2=512, H=A=1024).

Math (per batch b):
    P_lt = tanh(reps_lt[b] @ W) * diag_W        [L1, A]
    P_rt = tanh(reps_rt[b] @ W)                 [L2, A]
    S    = P_lt @ P_rt.T                        [L1, L2]
    out  = softmax(S, axis=-1)                  (masks are all-ones -> no-ops)

Distribution: data-parallel over batch across 8 NeuronCores (8 batches/core).

The kernel is PE-streaming-bound (~1280 matmuls x 512 moving columns =
655K cycles; measured at the PE streaming floor); everything else
overlaps.  This version squeezes the head/tail latency:
  - W and X are DMA'd in per-k-chunk contiguous tiles, with (w_k, x_lt_k)
    pairs interleaved at the head, and batch 0's lt projection runs
    k-outer across 4 PSUM banks so the PE consumes each 384 KB chunk the
    moment it lands instead of waiting for all of W;
  - dummy matmuls run during the DMA head so the PE HAM clock-gate is
    already lifted when real work starts;
  - softmax skips the max-subtraction (|S| <= sum|diag_W| ~ 26, so
    exp cannot overflow fp32) shortening the per-chunk chain;
  - output DMA is chunked per 128-row block instead of per batch.

Layout: the PE contracts over the partition dim, so both matmuls want
[contraction, free] operands.  reps are transposed to [H, L] on the host
during sharding; projections then produce P.T in [A, L] layout directly,
and the scores matmul consumes those tiles with no on-device transposes.
"""

from contextlib import ExitStack

import numpy as np

import concourse.bass as bass
import concourse.bacc as bacc
import concourse.mybir as mybir
import concourse.tile as tile
from concourse.bass_utils import run_bass_kernel_spmd

B, L, H, A = 64, 512, 1024, 1024
NCORES = 8
BPC = B // NCORES  # batches per core
PD = 128  # partition dim
KC = H // PD  # contraction chunks for proj
MC = A // PD  # att-dim chunks
LC = L // PD  # L1 chunks for scores

F32 = mybir.dt.float32
XDT = mybir.dt.bfloat16
WARMUP_MMS = 16  # ~1.7us; chunk-paced real matmuls continue the HAM activity


def _build_body(ctx: ExitStack, tc: "tile.TileContext", out, xt_lt, xt_rt, w, dw,
                repeat: int = 1):
    nc = tc.nc
    act = mybir.ActivationFunctionType

    wp = ctx.enter_context(tc.tile_pool(name="wpool", bufs=1))
    xp = ctx.enter_context(tc.tile_pool(name="xpool", bufs=2))
    pp = ctx.enter_context(tc.tile_pool(name="ppool", bufs=2))
    ep = ctx.enter_context(tc.tile_pool(name="epool", bufs=3))
    sp = ctx.enter_context(tc.tile_pool(name="spool", bufs=6))
    op = ctx.enter_context(tc.tile_pool(name="opool", bufs=4))
    ps_warm = ctx.enter_context(tc.tile_pool(name="psW", bufs=1, space="PSUM"))
    ps_proj = ctx.enter_context(tc.tile_pool(name="psA", bufs=4, space="PSUM"))
    ps_scr = ctx.enter_context(tc.tile_pool(name="psB", bufs=3, space="PSUM"))

    # PE warm-up: dummy matmuls on a zeroed tile, issued before any DMA
    # dependency so they run during the input-DMA head and lift the HAM
    # clock gate to 2.4 GHz.  Results land in a reserved PSUM bank that
    # nothing reads.
    warm = wp.tile([PD, PD], XDT)
    nc.vector.memset(warm, 0.0)
    wps = ps_warm.tile([PD, PD], F32)
    for i in range(WARMUP_MMS):
        nc.tensor.matmul(wps, lhsT=warm, rhs=warm, start=True, stop=True)

    # Head DMA order: (w_k, x_lt_k) pairs so the k-th projection chunk can
    # start as soon as ~((k+1) * 384 KB) has landed; x_rt and dw follow
    # (they are not needed until ~15 us in).
    w_sb = []
    first_xlt = []
    for k in range(KC):
        wk = wp.tile([PD, A], XDT, tag=f"w{k}")
        nc.sync.dma_start(out=wk, in_=w[k * PD:(k + 1) * PD, :])
        w_sb.append(wk)
        xl = xp.tile([PD, L], XDT, tag=f"xlt{k}")
        nc.sync.dma_start(out=xl, in_=xt_lt[0, k * PD:(k + 1) * PD, :])
        first_xlt.append(xl)
    dw_sb = wp.tile([PD, MC], F32)
    nc.sync.dma_start(out=dw_sb, in_=dw.rearrange("(m p) -> p m", p=PD))

    first = True
    for b in [bb for _ in range(repeat) for bb in range(BPC)]:
        # Per-k x tiles (contiguous [128, 512] DMAs); lt before rt so the
        # lt projection's chunk-k dependency clears as early as possible.
        if first:
            x_lt = first_xlt
        else:
            x_lt = []
            for k in range(KC):
                xl = xp.tile([PD, L], XDT, tag=f"xlt{k}")
                nc.sync.dma_start(out=xl, in_=xt_lt[b, k * PD:(k + 1) * PD, :])
                x_lt.append(xl)
        x_rt = []
        for k in range(KC):
            xr = xp.tile([PD, L], XDT, tag=f"xrt{k}")
            nc.sync.dma_start(out=xr, in_=xt_rt[b, k * PD:(k + 1) * PD, :])
            x_rt.append(xr)

        # Projections -> P.T tiles in [A, L] layout (m-chunk per tile).
        lt_p = [None] * MC
        rt_p = []
        if first:
            # k-outer in two half-sweeps of 4 m-chunks (= 4 PSUM banks):
            # each (w_k, x_lt_k) chunk is consumed the moment its DMA lands,
            # so the PE tracks the head DMA instead of waiting for all of W.
            for half in range(2):
                ms = range(half * 4, half * 4 + 4)
                pss = {
                    m: ps_proj.tile([PD, L], F32, tag="psp", name=f"ps_h{half}_{m}")
                    for m in ms
                }
                for k in range(KC):
                    for m in ms:
                        nc.tensor.matmul(
                            pss[m],
                            lhsT=w_sb[k][:, m * PD:(m + 1) * PD],
                            rhs=x_lt[k],
                            start=(k == 0),
                            stop=(k == KC - 1),
                        )
                for m in ms:
                    nc.scalar.activation(pss[m], pss[m], act.Tanh)
                    ltm = pp.tile([PD, L], XDT, tag=f"lt{m}")
                    nc.vector.tensor_scalar_mul(ltm, pss[m], dw_sb[:, m:m + 1])
                    lt_p[m] = ltm
            first = False
        else:
            for m in range(MC):
                ps = ps_proj.tile([PD, L], F32, tag="psp")
                for k in range(KC):
                    nc.tensor.matmul(
                        ps,
                        lhsT=w_sb[k][:, m * PD:(m + 1) * PD],
                        rhs=x_lt[k],
                        start=(k == 0),
                        stop=(k == KC - 1),
                    )
                # tanh in-place on PSUM, then scale by diag_W into SBUF.
                nc.scalar.activation(ps, ps, act.Tanh)
                ltm = pp.tile([PD, L], XDT, tag=f"lt{m}")
                nc.vector.tensor_scalar_mul(ltm, ps, dw_sb[:, m:m + 1])
                lt_p[m] = ltm
        for m in range(MC):
            ps = ps_proj.tile([PD, L], F32, tag="psp")
            for k in range(KC):
                nc.tensor.matmul(
                    ps,
                    lhsT=w_sb[k][:, m * PD:(m + 1) * PD],
                    rhs=x_rt[k],
                    start=(k == 0),
                    stop=(k == KC - 1),
                )
            rtm = pp.tile([PD, L], XDT, tag=f"rt{m}")
            nc.scalar.activation(rtm, ps, act.Tanh)
            rt_p.append(rtm)

        # Scores + row softmax (no max-subtraction: |S| <= sum|dw| ~ 26).
        for lm in range(LC):
            ps2 = ps_scr.tile([PD, L], F32, tag="pss")
            for m in range(MC):
                nc.tensor.matmul(
                    ps2,
                    lhsT=lt_p[m][:, lm * PD:(lm + 1) * PD],
                    rhs=rt_p[m],
                    start=(m == 0),
                    stop=(m == MC - 1),
                )
            ex = ep.tile([PD, L], F32, tag="ex")
            sm = sp.tile([PD, 1], F32, tag="sm")
            nc.scalar.activation(ex, ps2, act.Exp, accum_out=sm)
            rc = sp.tile([PD, 1], F32, tag="rc")
            nc.vector.reciprocal(rc, sm)
            o_sb = op.tile([PD, L], F32, tag="o")
            nc.vector.tensor_scalar_mul(o_sb, ex, rc)
            nc.sync.dma_start(out=out[b, lm * PD:(lm + 1) * PD, :], in_=o_sb)


def build_nc(repeat: int = 1) -> "bacc.Bacc":
    nc = bacc.Bacc("TRN2", target_bir_lowering=False, debug=False, num_devices=NCORES)
    xt_lt = nc.dram_tensor("xt_lt", [BPC, H, L], XDT, kind="ExternalInput").ap()
    xt_rt = nc.dram_tensor("xt_rt", [BPC, H, L], XDT, kind="ExternalInput").ap()
    w = nc.dram_tensor("w", [H, A], XDT, kind="ExternalInput").ap()
    dw = nc.dram_tensor("dw", [A], F32, kind="ExternalInput").ap()
    out = nc.dram_tensor("out", [BPC, L, L], F32, kind="ExternalOutput").ap()
    with tile.TileContext(nc) as tc, ExitStack() as ctx:
        _build_body(ctx, tc, out, xt_lt, xt_rt, w, dw, repeat=repeat)
    nc.compile()
    return nc


_NC_CACHE = None


def _get_nc():
    global _NC_CACHE
    if _NC_CACHE is None:
        _NC_CACHE = build_nc()
    return _NC_CACHE


def _x_np(a):
    import ml_dtypes

    return np.ascontiguousarray(a).astype(ml_dtypes.bfloat16)


def make_in_maps(reps_lt, reps_rt, attn_w1, diagonal_W):
    """Shard + lay out host-side: per-core [BPC, H, L] transposed inputs."""
    w = _x_np(np.asarray(attn_w1, dtype=np.float32))
    dw = np.ascontiguousarray(np.asarray(diagonal_W, dtype=np.float32).reshape(A))
    in_maps = []
    for c in range(NCORES):
        sl = slice(c * BPC, (c + 1) * BPC)
        xt_lt = _x_np(np.asarray(reps_lt[sl], dtype=np.float32).transpose(0, 2, 1))
        xt_rt = _x_np(np.asarray(reps_rt[sl], dtype=np.float32).transpose(0, 2, 1))
        in_maps.append({"xt_lt": xt_lt, "xt_rt": xt_rt, "w": w, "dw": dw})
    return in_maps


def kernel(reps_lt, reps_rt, mask_lt, mask_rt, attn_w1, diagonal_W):
    reps_lt = np.asarray(reps_lt, dtype=np.float32)
    reps_rt = np.asarray(reps_rt, dtype=np.float32)
    mask_lt = np.asarray(mask_lt, dtype=np.float32)
    mask_rt = np.asarray(mask_rt, dtype=np.float32)
    attn_w1 = np.asarray(attn_w1, dtype=np.float32)
    diagonal_W = np.asarray(diagonal_W, dtype=np.float32)

    if not (np.all(mask_lt == 1.0) and np.all(mask_rt == 1.0)):
        # General-mask fallback (never hit for this problem's all-ones masks):
        # multiplicative masking changes the softmax input, so compute on host.
        attn_lt = np.tanh(reps_lt @ attn_w1) * diagonal_W.reshape(1, 1, A)
        attn_rt = np.tanh(reps_rt @ attn_w1)
        s = np.einsum("bla,bra->blr", attn_lt, attn_rt)
        s = s * mask_lt[:, :, None] * mask_rt[:, None, :]
        e = np.exp(s - s.max(-1, keepdims=True))
        p = e / e.sum(-1, keepdims=True)
        return (p * mask_lt[:, :, None] * mask_rt[:, None, :]).astype(np.float32)

    nc = _get_nc()
    in_maps = make_in_maps(reps_lt, reps_rt, attn_w1, diagonal_W)
    res = run_bass_kernel_spmd(nc, in_maps, core_ids=list(range(NCORES)))
    out = np.concatenate([res.results[c]["out"] for c in range(NCORES)], axis=0)
    return np.ascontiguousarray(out.astype(np.float32))
